# revision 66
# baseline (speedup 1.0000x reference)
"""Trainium2 Bass kernel for complex (2-component) BatchNorm2d whitening.

Reference computation (per channel c):
    mu      = mean over (B, H, W) of z[..., :]            # [2]
    sigma   = cov over (B, H, W) of z (2x2) + eps*I       # [2, 2]
    W       = inv(sqrtm(sigma))                           # closed form 2x2
    o       = gamma @ (W @ (z - mu)) + beta

Strategy: shard the 64 channels across 8 NeuronCores (8 channels/core).
All reductions are over (B, H, W) which is fully local per channel, so
there is no cross-core communication at all.  Per channel the data
(4 MiB) is loaded into SBUF once; raw-moment statistics are computed
from SBUF with fused accumulating ops; the tiny 2x2 inverse-sqrt math
runs on one partition; the affine apply o = A*z + b (A = gamma@W,
b = beta - A@mu) reads the same SBUF-resident tiles and streams the
result back out.  HBM traffic is the minimum 2 passes (1 read + 1
write) = 64 MiB/core, ~186 us at ~360 GB/s.

Engine balance (stats at FD=2048/component, apply at FD=1024):
    ACT : Copy+accum S0/S1, Square+accum Q00/Q11, Identity(A01*z1+b0)
    DVE : scalar_tensor_tensor+accum for Q01, tensor_scalar A11*z1+b1,
          scalar_tensor_tensor fuses o0/o1, tiny 2x2 whitening math
    POOL: SWDGE stores + (tail channels) the o1 = v1+u1 add
    PE  : per-channel stats partition-reduce + A/b broadcast matmuls
    DMA : 2 MiB loads on the SP HWDGE ring, 1 MiB stores via SWDGE
All compute engines sit below the ~186 us/core DMA roofline; the
TimelineSim cost model puts the DMA device floor at 186,435 ns/core
plus ~1,966 ns startup (sem-clear preamble + HWDGE + DGE latency) and
~1,500 ns final drain (DMA sem prop + engine drains), i.e. ~189.9 us.

Tail scheduling (the last ~2 channels would otherwise expose their
stats->2x2->apply chain as DMA idle once the final load lands):
  * ts_split/split_stats_last=2: the last subtile's S0/S1 sums of the
    last two channels move to DVE tensor_scalar+accum (2x perf mode),
    splitting the exposed stats chain across ACT and DVE.
  * act_u1_last=2: u1 moves to ACT for the last two channels so DVE's
    apply burst (the o0/o1 fuses) shortens.
  * o1_pool=2 (skip final): for the 2 channels before the last one,
    o1 = (A10*z0 on DVE) + u1 on the GPSIMD engine - same expression
    tree, bit-identical - halving DVE's apply backlog so the final
    channel's chain starts earlier in DVE's in-order queue.
  * osplit_last=4: the last two channels store in 0.5 MiB groups, so
    their first stores are ready ~1 us earlier and flow at a finer
    cadence than the 2.9 us full-group transfers.
  * tail_sync_stores=3: the last three channels' stores ride the SP
    HWDGE ring (~1.3 us post-ready latency) instead of SWDGE
    (~2.1 us), and keep Pool's sequencer free for the o1 adds.
  * ga_psum: ACT's stats garbage output lives in 4 idle PSUM banks.
  * acc_reduce: the per-channel partition-reduce runs as one
    PSUM-accumulating ones-matmul per subtile (start/stop flags), so
    the first reduce overlaps the next subtile's load and the 2x2
    chain drops its S-combine step.
Result: 190,276 ns modeled (DMA gaps 11.3 us -> 0.34 us vs the
197,709 ns baseline; floor is ~189.9 us), hardware rel-err 3.2e-7.
"""

import sys

if "/opt/trn_rl_repo" not in sys.path:
    sys.path.insert(0, "/opt/trn_rl_repo")

from contextlib import ExitStack

import numpy as np

import concourse.bass as bass
import concourse.tile as tile
from concourse import bacc, mybir

N_CORES = 8
B, C, H, W = 32, 64, 128, 128
C_LOC = C // N_CORES
EPS = 1e-5

F32 = mybir.dt.float32
AF = mybir.ActivationFunctionType
OP = mybir.AluOpType

# Tuned pipeline configuration (TimelineSim A/B results; see transcript).
# ts_split+split_stats_last=2: last-subtile S0/S1 of the last two channels
# move to DVE tensor_scalar copy+accum (2x perf mode) so the tail stats
# chain is split across ACT/DVE; act_u1_last=2: u1 on ACT for the last two
# channels to unload DVE's apply burst; reserve=1+ga_psum: ACT stats
# garbage lives in PSUM, freeing SBUF for one deferred first-channel store
# that fills the post-load tail gap.
CFG = dict(half_b=16, osplit=2, ld_split=1, zbufs=8, obufs=5, ubufs=6,
           load_engs=("sync",), store_engs=("gpsimd",),
           ts_split=1, split_stats_last=2, act_u1_last=2, ga_psum=1,
           osplit_last=4, osl_chans=2, o1_pool=2, o1_pool_skipf=1,
           tail_sync_stores=3, acc_reduce=1)


def build_program(b, c_loc, h, w, half_b=16, osplit=2, zbufs=7, obufs=3,
                  ubufs=3, load_engs=("sync",),
                  store_engs=("gpsimd",), ld_split=1, repeat=1,
                  dma_block=0, probe="full", last_half_b=None, dve_u0_last=0,
                  split_stats_last=0, act_u1_last=0, tail_sync_stores=1,
                  s_dve=0, u1_act=0, reserve=0, ts_split=0, inplace=0,
                  stats_alt=0, o1_pool=0, ga_psum=0, remit=None,
                  q01_pool=0, defer_prev=0, osplit_last=0, osl_chans=2,
                  o1_pool_skipf=0, defer_prev2=0, v1_act=0, acc_reduce=0,
                  ab_psum=0, dve_u0_o1p=0):
    """Build the per-core Bass program.  Shapes parameterized for sim tests.

    half_b : batch rows per z/stats sub-tile (b/half_b sub-tiles per channel)
    osplit : apply/store sub-tiles per z sub-tile
    ld_split: DMA transfers per z sub-tile load
    """
    def tiling(hb):
        assert b % hb == 0 and hb % osplit == 0 and hb % ld_split == 0
        return hb, b // hb, hb // osplit, hb // ld_split

    inv_n = 1.0 / float(b * h * w)

    nc = bacc.Bacc("TRN2", target_bir_lowering=False, debug=False,
                   num_devices=N_CORES)
    z_ap = nc.dram_tensor("z", [b, c_loc, h, w, 2], F32, kind="ExternalInput").ap()
    g_ap = nc.dram_tensor("gamma", [1, 4], F32, kind="ExternalInput").ap()
    be_ap = nc.dram_tensor("beta", [1, 2], F32, kind="ExternalInput").ap()
    o_ap = nc.dram_tensor("out", [b, c_loc, h, w, 2], F32, kind="ExternalOutput").ap()

    def eng(name):
        return {"sync": nc.sync, "scalar": nc.scalar, "gpsimd": nc.gpsimd,
                "vector": nc.vector}[name]

    load_cycle = [eng(e) for e in load_engs]
    store_cycle = [eng(e) for e in store_engs]

    with tile.TileContext(nc) as tc, ExitStack() as ctx:
        consts = ctx.enter_context(tc.tile_pool(name="consts", bufs=1))
        zpool = ctx.enter_context(tc.tile_pool(name="z", bufs=zbufs))
        opool = (None if inplace else
                 ctx.enter_context(tc.tile_pool(name="o", bufs=obufs)))
        upool = ctx.enter_context(tc.tile_pool(name="u", bufs=ubufs))
        gapool = ctx.enter_context(tc.tile_pool(
            name="ga", bufs=1, **(dict(space="PSUM") if ga_psum else {})))
        gdpool = ctx.enter_context(tc.tile_pool(name="gd", bufs=1))
        stpool = ctx.enter_context(tc.tile_pool(name="st", bufs=2))
        rpool = (ctx.enter_context(tc.tile_pool(name="r", bufs=reserve))
                 if reserve else None)
        abpool = ctx.enter_context(tc.tile_pool(name="ab", bufs=2))
        tpool = ctx.enter_context(tc.tile_pool(name="tiny", bufs=2))
        pspool = ctx.enter_context(tc.tile_pool(name="ps", bufs=2, space="PSUM"))
        bcpool = ctx.enter_context(tc.tile_pool(name="bc", bufs=2, space="PSUM"))

        # constants
        ones_col = consts.tile([h, 1], F32, tag="ones_col")
        nc.vector.memset(ones_col[:], 1.0)
        ones_row = consts.tile([1, h], F32, tag="ones_row")
        nc.vector.memset(ones_row[:], 1.0)
        eps3 = consts.tile([1, 3], F32, tag="eps3")
        nc.vector.memset(eps3[:, 0:1], EPS)
        nc.vector.memset(eps3[:, 1:2], 0.0)
        nc.vector.memset(eps3[:, 2:3], EPS)
        zero_col = consts.tile([h, 1], F32, tag="zero_col")
        nc.vector.memset(zero_col[:], 0.0)
        # gamma/beta ride the otherwise-idle ACT HWDGE ring so the first
        # z load is the first transfer on the SP ring
        gsb = consts.tile([1, 4], F32, tag="gsb")
        nc.scalar.dma_start(gsb[:], g_ap[:])
        bsb = consts.tile([1, 2], F32, tag="bsb")
        nc.scalar.dma_start(bsb[:], be_ap[:])

        n_dma = 0
        deferred = []  # (dst, tile) store pairs held to the kernel end
        chans = [cc for _ in range(repeat) for cc in range(c_loc)]
        for idx, c in enumerate(chans):
            hb_c = (last_half_b if (last_half_b and idx == len(chans) - 1)
                    else half_b)
            half_b_c, n_half_c, qb_c, lb_c = tiling(hb_c)
            st = stpool.tile([h, 5 * n_half_c], F32, tag="st")
            ps_acc = None
            if acc_reduce:
                ps_acc = pspool.tile([1, 5], F32, tag="psa")
            z_tiles = []
            # ---- pass 1: load + statistics -------------------------------
            for hf in range(n_half_c):
                # held subtiles of the first channel live in rpool until the
                # kernel end (in-place mode: the z tile becomes the o tile)
                zhold = (inplace and reserve and idx == 0
                         and hf >= n_half_c - reserve)
                zt = (rpool if zhold else zpool).tile(
                    [h, half_b_c, w, 2], F32, tag="zr" if zhold else "zt")
                src = z_ap[:, c].transpose([1, 0, 2, 3])[
                    :, hf * half_b_c:(hf + 1) * half_b_c]
                if dma_block:
                    # one transfer per dma_block batch rows: each is a fully
                    # contiguous DRAM run, issued in sequential DRAM order
                    for bb in range(0, half_b_c, dma_block):
                        ls = slice(bb, bb + dma_block)
                        load_cycle[n_dma % len(load_cycle)].dma_start(
                            zt[:, ls], src[:, ls])
                        n_dma += 1
                else:
                    for l in range(ld_split):
                        ls = slice(l * lb_c, (l + 1) * lb_c)
                        load_cycle[n_dma % len(load_cycle)].dma_start(
                            zt[:, ls], src[:, ls])
                        n_dma += 1
                z_tiles.append(zt)
                if probe == "dma":
                    continue
                r_idx, r_hf = remit if remit else (len(chans) - 1,
                                                  n_half_c - 1)
                if ((reserve or defer_prev or defer_prev2)
                        and idx == r_idx and hf == r_hf):
                    # Release the deferred first-channel stores.  Emission
                    # position controls the conservative cross-engine wait
                    # the framework attaches, i.e. when the transfer lands;
                    # tuned so it fills the tail gap after the final load.
                    for ddst, dot in deferred:
                        nc.sync.dma_start(ddst, dot[:])
                        n_dma += 1
                    deferred = []
                z0 = zt[:, :, :, 0]
                z1 = zt[:, :, :, 1]
                o5 = 5 * hf
                ga = gapool.tile([h, half_b_c, w], F32, tag="ga")
                tail = (idx >= len(chans) - split_stats_last
                        and hf == n_half_c - 1)
                if s_dve or (stats_alt and hf % 2 == 1):
                    # S0/S1 on DVE tensor_scalar copy+accum (2x perf mode:
                    # ~0.55 ns/elem vs 0.92 on ACT).  stats_alt: alternate
                    # subtiles so the channel's stats finish ~equally early
                    # on both engines (shortens the stats->apply latency).
                    gd2 = gdpool.tile([h, half_b_c, w], F32, tag="gd")
                    nc.vector.tensor_scalar(
                        out=gd2[:], in0=z0, scalar1=1.0, scalar2=0.0,
                        op0=OP.mult, op1=OP.add,
                        accum_out=st[:, o5 + 0:o5 + 1])
                    nc.vector.tensor_scalar(
                        out=gd2[:], in0=z1, scalar1=1.0, scalar2=0.0,
                        op0=OP.mult, op1=OP.add,
                        accum_out=st[:, o5 + 1:o5 + 2])
                elif tail:
                    # last-loaded subtile: split the 4 accumulations across
                    # ACT and DVE so the exposed tail chain halves.
                    # ts_split uses tensor_scalar (2x DVE perf mode) instead
                    # of scalar_tensor_tensor (1x).
                    gd2 = gdpool.tile([h, half_b_c, w], F32, tag="gd")
                    if ts_split:
                        # two-op form: walrus requires a 2nd op for
                        # TensorScalarPtrReduce (accum) variants
                        nc.vector.tensor_scalar(
                            out=gd2[:], in0=z0, scalar1=1.0, scalar2=0.0,
                            op0=OP.mult, op1=OP.add,
                            accum_out=st[:, o5 + 0:o5 + 1])
                        nc.vector.tensor_scalar(
                            out=gd2[:], in0=z1, scalar1=1.0, scalar2=0.0,
                            op0=OP.mult, op1=OP.add,
                            accum_out=st[:, o5 + 1:o5 + 2])
                    else:
                        nc.vector.scalar_tensor_tensor(
                            out=gd2[:], in0=z0, scalar=1.0, in1=z1,
                            op0=OP.mult, op1=OP.bypass,
                            accum_out=st[:, o5 + 0:o5 + 1])
                        nc.vector.scalar_tensor_tensor(
                            out=gd2[:], in0=z1, scalar=1.0, in1=z0,
                            op0=OP.mult, op1=OP.bypass,
                            accum_out=st[:, o5 + 1:o5 + 2])
                else:
                    nc.scalar.activation(ga[:], z0, AF.Copy,
                                         accum_out=st[:, o5 + 0:o5 + 1])
                    nc.scalar.activation(ga[:], z1, AF.Copy,
                                         accum_out=st[:, o5 + 1:o5 + 2])
                nc.scalar.activation(ga[:], z0, AF.Square,
                                     accum_out=st[:, o5 + 2:o5 + 3])
                nc.scalar.activation(ga[:], z1, AF.Square,
                                     accum_out=st[:, o5 + 4:o5 + 5])
                # Q01 = sum(z0*z1): out = (z0 bypass s) mult z1, accum = sum.
                # (tensor_tensor_reduce crashes this runtime; this is the
                # hardware-verified equivalent.)  q01_pool shifts it to the
                # mostly-idle GPSIMD engine for the channels just before the
                # final one, unloading DVE's tail cascade (the final
                # channel's Q01 stays on DVE: Pool latency would lengthen
                # the exposed tail chain).
                gd = gdpool.tile([h, half_b_c, w], F32, tag="gd")
                q01_eng = (nc.gpsimd if (idx >= len(chans) - 1 - q01_pool
                                         and idx != len(chans) - 1)
                           else nc.vector)
                q01_eng.scalar_tensor_tensor(
                    out=gd[:], in0=z0, scalar=0.0, in1=z1,
                    op0=OP.bypass, op1=OP.mult,
                    accum_out=st[:, o5 + 3:o5 + 4])
                if acc_reduce and probe == "full":
                    # per-subtile partition-reduce, PSUM-accumulated: the
                    # first matmul overlaps the next subtile's load, and the
                    # 2x2 chain loses its S-combine step
                    nc.tensor.matmul(ps_acc[:], lhsT=ones_col[:],
                                     rhs=st[:, o5:o5 + 5],
                                     start=(hf == 0),
                                     stop=(hf == n_half_c - 1))

            if probe in ("dma", "stats"):
                # store straight from the input tiles (timing probe only)
                for hf in range(n_half_c):
                    dst = o_ap[:, c].transpose([1, 0, 2, 3])[
                        :, hf * half_b_c:(hf + 1) * half_b_c]
                    store_cycle[n_dma % len(store_cycle)].dma_start(
                        dst, z_tiles[hf][:])
                    n_dma += 1
                continue

            # ---- partition-reduce the stats with a ones-matmul -----------
            if not acc_reduce:
                ps = pspool.tile([1, 5 * n_half_c], F32, tag="ps")
                nc.tensor.matmul(ps[:], lhsT=ones_col[:], rhs=st[:],
                                 start=True, stop=True)

            # ---- tiny per-channel 2x2 math on partition 0 ----------------
            # T layout (offsets into [1, 64]):
            #  0:5 S | 5:7 mu | 7:10 prods | 13:16 cov | 16 d1 17 d2 18 det
            #  19 s | 20 tr | 21 tr2s | 22 t | 23:26 numer | 26 dsn1 27 dsn2
            #  28 dsn | 29 rdn | 30 f | 31 fn | 32:36 W | 36:40 tmp
            #  40:46 AB = (A00 A01 A10 A11 b0 b1) | 48:52 prod4 | 52:54 ps2
            #  54:64 spare (S-combine accumulators) | 64:64+5n raw psum copy
            T = tpool.tile([1, 112], F32, tag="T")
            v = nc.vector

            def tt(dst, a, bb, op):
                v.tensor_tensor(out=dst, in0=a, in1=bb, op=op)

            # PSUM row -> SBUF (HW allows only one PSUM operand per inst)
            S = T[:, 0:5]
            if acc_reduce:
                # subtile groups were already summed in PSUM by the
                # accumulating matmuls; a single 5-wide copy lands S
                v.tensor_copy(S, ps_acc[:])
            else:
                P = T[:, 64:64 + 5 * n_half_c]
                v.tensor_copy(P, ps[:])

            # S = sum of per-subtile stats groups
            if acc_reduce:
                pass
            elif n_half_c == 1:
                v.tensor_copy(S, P[:, 0:5])
            elif n_half_c == 2:
                tt(S, P[:, 0:5], P[:, 5:10], OP.add)
            else:
                acc = [T[:, 54:59], T[:, 59:64]]
                tt(acc[0], P[:, 0:5], P[:, 5:10], OP.add)
                cur = 0
                for k in range(2, n_half_c):
                    dst = S if k == n_half_c - 1 else acc[1 - cur]
                    tt(dst, acc[cur], P[:, 5 * k:5 * k + 5], OP.add)
                    cur = 1 - cur
            v.tensor_scalar(out=T[:, 5:7], in0=T[:, 0:2], scalar1=inv_n,
                            scalar2=None, op0=OP.mult)
            # prods = (mu0^2, mu0*mu1, mu1^2)
            tt(T[:, 7:9], T[:, 5:7], T[:, 5:6].broadcast_to([1, 2]), OP.mult)
            tt(T[:, 9:10], T[:, 6:7], T[:, 6:7], OP.mult)
            # cov = Q*inv_n - prods + eps*I
            v.scalar_tensor_tensor(out=T[:, 10:13], in0=T[:, 2:5], scalar=inv_n,
                                   in1=T[:, 7:10], op0=OP.mult, op1=OP.subtract)
            tt(T[:, 13:16], T[:, 10:13], eps3[:, 0:3], OP.add)
            # det & s = sqrt(det)
            tt(T[:, 16:17], T[:, 13:14], T[:, 15:16], OP.mult)
            tt(T[:, 17:18], T[:, 14:15], T[:, 14:15], OP.mult)
            tt(T[:, 18:19], T[:, 16:17], T[:, 17:18], OP.subtract)
            nc.scalar.activation(T[:, 19:20], T[:, 18:19], AF.Sqrt)
            # t = sqrt(trace + 2s)
            tt(T[:, 20:21], T[:, 13:14], T[:, 15:16], OP.add)
            v.scalar_tensor_tensor(out=T[:, 21:22], in0=T[:, 19:20], scalar=2.0,
                                   in1=T[:, 20:21], op0=OP.mult, op1=OP.add)
            nc.scalar.activation(T[:, 22:23], T[:, 21:22], AF.Sqrt)
            # numer = (c00+s, c01, c11+s);  W = t/det(numer) * adj(numer)
            tt(T[:, 23:26:2], T[:, 13:16:2], T[:, 19:20].broadcast_to([1, 2]),
               OP.add)
            v.tensor_copy(T[:, 24:25], T[:, 14:15])
            tt(T[:, 26:27], T[:, 23:24], T[:, 25:26], OP.mult)
            tt(T[:, 27:28], T[:, 24:25], T[:, 24:25], OP.mult)
            tt(T[:, 28:29], T[:, 26:27], T[:, 27:28], OP.subtract)
            v.reciprocal(T[:, 29:30], T[:, 28:29])
            tt(T[:, 30:31], T[:, 22:23], T[:, 29:30], OP.mult)
            v.tensor_scalar(out=T[:, 31:32], in0=T[:, 30:31], scalar1=-1.0,
                            scalar2=None, op0=OP.mult)
            # W = (W00, W01, W10, W11) = (f*n2, fn*n1, fn*n1, f*n0)
            tt(T[:, 32:33], T[:, 25:26], T[:, 30:31], OP.mult)
            tt(T[:, 33:34], T[:, 24:25], T[:, 31:32], OP.mult)
            v.tensor_copy(T[:, 34:35], T[:, 33:34])
            tt(T[:, 35:36], T[:, 23:24], T[:, 30:31], OP.mult)
            # A = gamma @ W  (row i = g_i0 * Wtop + g_i1 * Wbot)
            v.tensor_scalar(out=T[:, 36:38], in0=T[:, 32:34],
                            scalar1=gsb[:, 0:1], scalar2=None, op0=OP.mult)
            v.scalar_tensor_tensor(out=T[:, 40:42], in0=T[:, 34:36],
                                   scalar=gsb[:, 1:2], in1=T[:, 36:38],
                                   op0=OP.mult, op1=OP.add)
            v.tensor_scalar(out=T[:, 38:40], in0=T[:, 32:34],
                            scalar1=gsb[:, 2:3], scalar2=None, op0=OP.mult)
            v.scalar_tensor_tensor(out=T[:, 42:44], in0=T[:, 34:36],
                                   scalar=gsb[:, 3:4], in1=T[:, 38:40],
                                   op0=OP.mult, op1=OP.add)
            # b = beta - A @ mu
            tt(T[:, 48:50], T[:, 40:42], T[:, 5:7], OP.mult)
            tt(T[:, 50:52], T[:, 42:44], T[:, 5:7], OP.mult)
            tt(T[:, 52:54], T[:, 48:52:2], T[:, 49:52:2], OP.add)
            tt(T[:, 44:46], bsb[:, 0:2], T[:, 52:54], OP.subtract)

            # ---- broadcast (A, b) to all partitions via K=1 matmul -------
            bc = bcpool.tile([h, 6], F32, tag="bc")
            nc.tensor.matmul(bc[:], lhsT=ones_row[:], rhs=T[:, 40:46],
                             start=True, stop=True)
            if ab_psum:
                # apply ops read the per-partition (A, b) scalars straight
                # from the PSUM broadcast (their only PSUM operand), saving
                # the SBUF copy on the chain critical path
                ab = bc
            else:
                ab = abpool.tile([h, 6], F32, tag="ab")
                v.tensor_copy(ab[:], bc[:])

            # ---- pass 2: apply o = A z + b, store ------------------------
            osplit_c = (osplit_last
                        if (osplit_last and idx >= len(chans) - osl_chans)
                        else osplit)
            qb_c = half_b_c // osplit_c
            for hf in range(n_half_c):
                zt = z_tiles[hf]
                for q in range(osplit_c):
                    bs = slice(q * qb_c, (q + 1) * qb_c)
                    z0 = zt[:, bs, :, 0]
                    z1 = zt[:, bs, :, 1]
                    u0 = upool.tile([h, qb_c, w], F32, tag="u")
                    u1 = upool.tile([h, qb_c, w], F32, tag="u")
                    o1p = (not inplace
                           and (idx >= len(chans) - 1 - o1_pool
                                and idx != len(chans) - 1
                                if o1_pool_skipf
                                else idx >= len(chans) - o1_pool))

                    # u0 = A01*z1 + b0 (ACT; DVE for the tail channels so
                    # the kernel end is not gated on the busiest engine)
                    def emit_u0():
                        if (idx >= len(chans) - dve_u0_last
                                or (dve_u0_o1p and o1p)):
                            nc.vector.tensor_scalar(
                                out=u0[:], in0=z1, scalar1=ab[:, 1:2],
                                scalar2=ab[:, 4:5], op0=OP.mult, op1=OP.add)
                        else:
                            nc.scalar.activation(u0[:], z1, AF.Identity,
                                                 bias=ab[:, 4:5],
                                                 scale=ab[:, 1:2])

                    def emit_u1():
                        if u1_act or idx >= len(chans) - act_u1_last:
                            # DVE is the apply-chain bottleneck (u1 + 2
                            # fuses); produce u1 on ACT instead
                            nc.scalar.activation(u1[:], z1, AF.Identity,
                                                 bias=ab[:, 5:6],
                                                 scale=ab[:, 3:4])
                        else:
                            nc.vector.tensor_scalar(
                                out=u1[:], in0=z1, scalar1=ab[:, 3:4],
                                scalar2=ab[:, 5:6], op0=OP.mult, op1=OP.add)

                    emit_u0()
                    emit_u1()
                    if inplace:
                        # Apply writes back into the z tile: z1 is dead once
                        # u0/u1 are computed, so o1 overwrites it; z0 is dead
                        # once o1 has read it, so o0 overwrites z0 last.
                        hold = (reserve and idx == 0
                                and hf >= n_half_c - reserve)
                        ot = zt[:, bs]
                        nc.vector.scalar_tensor_tensor(
                            out=zt[:, bs, :, 1], in0=z0, scalar=ab[:, 2:3],
                            in1=u1[:], op0=OP.mult, op1=OP.add)
                        nc.vector.scalar_tensor_tensor(
                            out=zt[:, bs, :, 0], in0=z0, scalar=ab[:, 0:1],
                            in1=u0[:], op0=OP.mult, op1=OP.add)
                    else:
                        g_idx = hf * osplit + q
                        hold = (reserve and idx == 0
                                and g_idx >= n_half_c * osplit - reserve)
                        ot = (rpool if hold else opool).tile(
                            [h, qb_c, w, 2], F32, tag="rt" if hold else "ot")
                        if o1p:
                            # o1 = (A10*z0) + u1 with the final add on the
                            # mostly-idle GPSIMD engine; same expression tree
                            # as the fused stt so numerics are unchanged.
                            # v1_act computes A10*z0 on ACT (Identity with a
                            # zero bias) instead of a DVE tensor_scalar.
                            v1 = upool.tile([h, qb_c, w], F32, tag="u")
                            if v1_act:
                                nc.scalar.activation(
                                    v1[:], z0, AF.Identity,
                                    bias=zero_col[:, 0:1],
                                    scale=ab[:, 2:3])
                            else:
                                nc.vector.tensor_scalar(
                                    out=v1[:], in0=z0, scalar1=ab[:, 2:3],
                                    scalar2=None, op0=OP.mult)
                            nc.gpsimd.tensor_tensor(
                                out=ot[:, :, :, 1], in0=v1[:], in1=u1[:],
                                op=OP.add)
                        else:
                            nc.vector.scalar_tensor_tensor(
                                out=ot[:, :, :, 1], in0=z0, scalar=ab[:, 2:3],
                                in1=u1[:], op0=OP.mult, op1=OP.add)
                        nc.vector.scalar_tensor_tensor(
                            out=ot[:, :, :, 0], in0=z0, scalar=ab[:, 0:1],
                            in1=u0[:], op0=OP.mult, op1=OP.add)
                    dst = o_ap[:, c].transpose([1, 0, 2, 3])[
                        :, hf * half_b_c + q * qb_c:hf * half_b_c + (q + 1) * qb_c]
                    ssrc = ot if inplace else ot[:]
                    if not hold and defer_prev and idx == len(chans) - 2:
                        # Defer the second-to-last channel's trailing stores:
                        # emitted after the final channel's loads so their
                        # SemWaits cannot block the last load decodes on the
                        # SP sequencer, yet their (ready) data fills the DMA
                        # pipe the moment the final load transfer ends.
                        g_idx = hf * osplit_c + q
                        hold = g_idx >= n_half_c * osplit_c - defer_prev
                    if not hold and defer_prev2 and idx == len(chans) - 3:
                        # same for the third-to-last channel's trailing
                        # stores: their SWDGE desc-gen latency (~2.1us) was
                        # exposing a mid-tail bubble; the sync-ring deferred
                        # path re-queues them ~0.8us earlier
                        g_idx = hf * osplit_c + q
                        hold = g_idx >= n_half_c * osplit_c - defer_prev2
                    if hold:
                        assert not inplace, "reserve requires tile-held o"
                        deferred.append((dst, ot))
                        continue
                    if dma_block:
                        for bb in range(0, qb_c, dma_block):
                            os_ = slice(bb, bb + dma_block)
                            store_cycle[n_dma % len(store_cycle)].dma_start(
                                dst[:, os_], ot[:, os_])
                            n_dma += 1
                    else:
                        # last channel: loads are done, the SP HWDGE ring is
                        # free and issues ~2us faster than SWDGE descriptors
                        if idx >= len(chans) - tail_sync_stores:
                            nc.sync.dma_start(dst, ssrc)
                        else:
                            store_cycle[n_dma % len(store_cycle)].dma_start(
                                dst, ssrc)
                        n_dma += 1

    nc.compile()
    return nc


_PROGRAM_CACHE = {}


def _get_program(key):
    if key not in _PROGRAM_CACHE:
        _PROGRAM_CACHE[key] = build_program(*key[:4], **dict(key[4]))
    return _PROGRAM_CACHE[key]


def kernel(z, gamma, beta):
    z = np.asarray(z, dtype=np.float32)
    gamma = np.asarray(gamma, dtype=np.float32)
    beta = np.asarray(beta, dtype=np.float32)
    assert z.shape == (B, C, H, W, 2), z.shape

    from concourse.bass_utils import run_bass_kernel_spmd

    nc = _get_program((B, C_LOC, H, W, tuple(sorted(CFG.items()))))

    g4 = np.ascontiguousarray(gamma.reshape(1, 4))
    b2 = np.ascontiguousarray(beta.reshape(1, 2))
    in_maps = []
    for k in range(N_CORES):
        shard = np.ascontiguousarray(z[:, k * C_LOC:(k + 1) * C_LOC])
        in_maps.append({"z": shard, "gamma": g4, "beta": b2})

    res = run_bass_kernel_spmd(nc, in_maps, list(range(N_CORES)))
    out = np.concatenate([res.results[k]["out"] for k in range(N_CORES)], axis=1)
    return out



# revision 70
# speedup vs baseline: 1.0016x; 1.0016x over previous
"""Trainium2 Bass kernel for complex (2-component) BatchNorm2d whitening.

Reference computation (per channel c):
    mu      = mean over (B, H, W) of z[..., :]            # [2]
    sigma   = cov over (B, H, W) of z (2x2) + eps*I       # [2, 2]
    W       = inv(sqrtm(sigma))                           # closed form 2x2
    o       = gamma @ (W @ (z - mu)) + beta

Strategy: shard the 64 channels across 8 NeuronCores (8 channels/core).
All reductions are over (B, H, W) which is fully local per channel, so
there is no cross-core communication at all.  Per channel the data
(4 MiB) is loaded into SBUF once; raw-moment statistics are computed
from SBUF with fused accumulating ops; the tiny 2x2 inverse-sqrt math
runs on one partition; the affine apply o = A*z + b (A = gamma@W,
b = beta - A@mu) reads the same SBUF-resident tiles and streams the
result back out.  HBM traffic is the minimum 2 passes (1 read + 1
write) = 64 MiB/core, ~186 us at ~360 GB/s.

Engine balance (stats at FD=2048/component, apply at FD=1024):
    ACT : Copy+accum S0/S1, Square+accum Q00/Q11, Identity(A01*z1+b0)
    DVE : scalar_tensor_tensor+accum for Q01, tensor_scalar A11*z1+b1,
          scalar_tensor_tensor fuses o0/o1, tiny 2x2 whitening math
    POOL: SWDGE stores + (tail channels) the o1 = v1+u1 add
    PE  : per-channel stats partition-reduce + A/b broadcast matmuls
    DMA : 2 MiB loads on the SP HWDGE ring, 1 MiB stores via SWDGE
All compute engines sit below the ~186 us/core DMA roofline; the
TimelineSim cost model puts the DMA device floor at 186,435 ns/core
plus ~1,966 ns startup (sem-clear preamble + HWDGE + DGE latency) and
~1,500 ns final drain (DMA sem prop + engine drains), i.e. ~189.9 us.

Tail scheduling (the last ~2 channels would otherwise expose their
stats->2x2->apply chain as DMA idle once the final load lands):
  * ts_split/split_stats_last=2: the last subtile's S0/S1 sums of the
    last two channels move to DVE tensor_scalar+accum (2x perf mode),
    splitting the exposed stats chain across ACT and DVE.
  * act_u1_last=2: u1 moves to ACT for the last two channels so DVE's
    apply burst (the o0/o1 fuses) shortens.
  * o1_pool=2 (skip final): for the 2 channels before the last one,
    o1 = (A10*z0 on DVE) + u1 on the GPSIMD engine - same expression
    tree, bit-identical - halving DVE's apply backlog so the final
    channel's chain starts earlier in DVE's in-order queue.
  * osplit_last=4: the last two channels store in 0.5 MiB groups, so
    their first stores are ready ~1 us earlier and flow at a finer
    cadence than the 2.9 us full-group transfers.
  * tail_sync_stores=3: the last three channels' stores ride the SP
    HWDGE ring (~1.3 us post-ready latency) instead of SWDGE
    (~2.1 us), and keep Pool's sequencer free for the o1 adds.
  * ga_psum: ACT's stats garbage output lives in 4 idle PSUM banks.
  * acc_reduce: the per-channel partition-reduce runs as one
    PSUM-accumulating ones-matmul per subtile (start/stop flags), so
    the first reduce overlaps the next subtile's load and the 2x2
    chain drops its S-combine step.
Result: 190,276 ns modeled (DMA gaps 11.3 us -> 0.34 us vs the
197,709 ns baseline; floor is ~189.9 us), hardware rel-err 3.2e-7.
"""

import sys

if "/opt/trn_rl_repo" not in sys.path:
    sys.path.insert(0, "/opt/trn_rl_repo")

from contextlib import ExitStack

import numpy as np

import concourse.bass as bass
import concourse.tile as tile
from concourse import bacc, mybir

N_CORES = 8
B, C, H, W = 32, 64, 128, 128
C_LOC = C // N_CORES
EPS = 1e-5

F32 = mybir.dt.float32
AF = mybir.ActivationFunctionType
OP = mybir.AluOpType

# Tuned pipeline configuration (TimelineSim A/B results; see transcript).
# ts_split+split_stats_last=2: last-subtile S0/S1 of the last two channels
# move to DVE tensor_scalar copy+accum (2x perf mode) so the tail stats
# chain is split across ACT/DVE; act_u1_last=2: u1 on ACT for the last two
# channels to unload DVE's apply burst; reserve=1+ga_psum: ACT stats
# garbage lives in PSUM, freeing SBUF for one deferred first-channel store
# that fills the post-load tail gap.
CFG = dict(half_b=16, osplit=2, ld_split=1, zbufs=8, obufs=5, ubufs=6,
           load_engs=("sync",), store_engs=("gpsimd",),
           ts_split=1, split_stats_last=2, act_u1_last=2, ga_psum=1,
           osplit_last=4, osl_chans=2, o1_pool=2, o1_pool_skipf=1,
           tail_sync_stores=3, acc_reduce=1, fuse_g0=1, du0_g0=1)


def build_program(b, c_loc, h, w, half_b=16, osplit=2, zbufs=7, obufs=3,
                  ubufs=3, load_engs=("sync",),
                  store_engs=("gpsimd",), ld_split=1, repeat=1,
                  dma_block=0, probe="full", last_half_b=None, dve_u0_last=0,
                  split_stats_last=0, act_u1_last=0, tail_sync_stores=1,
                  s_dve=0, u1_act=0, reserve=0, ts_split=0, inplace=0,
                  stats_alt=0, o1_pool=0, ga_psum=0, remit=None,
                  q01_pool=0, defer_prev=0, osplit_last=0, osl_chans=2,
                  o1_pool_skipf=0, defer_prev2=0, v1_act=0, acc_reduce=0,
                  ab_psum=0, dve_u0_o1p=0, fuse_g0=0, du0_g0=0):
    """Build the per-core Bass program.  Shapes parameterized for sim tests.

    half_b : batch rows per z/stats sub-tile (b/half_b sub-tiles per channel)
    osplit : apply/store sub-tiles per z sub-tile
    ld_split: DMA transfers per z sub-tile load
    """
    def tiling(hb):
        assert b % hb == 0 and hb % osplit == 0 and hb % ld_split == 0
        return hb, b // hb, hb // osplit, hb // ld_split

    inv_n = 1.0 / float(b * h * w)

    nc = bacc.Bacc("TRN2", target_bir_lowering=False, debug=False,
                   num_devices=N_CORES)
    z_ap = nc.dram_tensor("z", [b, c_loc, h, w, 2], F32, kind="ExternalInput").ap()
    g_ap = nc.dram_tensor("gamma", [1, 4], F32, kind="ExternalInput").ap()
    be_ap = nc.dram_tensor("beta", [1, 2], F32, kind="ExternalInput").ap()
    o_ap = nc.dram_tensor("out", [b, c_loc, h, w, 2], F32, kind="ExternalOutput").ap()

    def eng(name):
        return {"sync": nc.sync, "scalar": nc.scalar, "gpsimd": nc.gpsimd,
                "vector": nc.vector}[name]

    load_cycle = [eng(e) for e in load_engs]
    store_cycle = [eng(e) for e in store_engs]

    with tile.TileContext(nc) as tc, ExitStack() as ctx:
        consts = ctx.enter_context(tc.tile_pool(name="consts", bufs=1))
        zpool = ctx.enter_context(tc.tile_pool(name="z", bufs=zbufs))
        opool = (None if inplace else
                 ctx.enter_context(tc.tile_pool(name="o", bufs=obufs)))
        upool = ctx.enter_context(tc.tile_pool(name="u", bufs=ubufs))
        gapool = ctx.enter_context(tc.tile_pool(
            name="ga", bufs=1, **(dict(space="PSUM") if ga_psum else {})))
        gdpool = ctx.enter_context(tc.tile_pool(name="gd", bufs=1))
        stpool = ctx.enter_context(tc.tile_pool(name="st", bufs=2))
        rpool = (ctx.enter_context(tc.tile_pool(name="r", bufs=reserve))
                 if reserve else None)
        abpool = ctx.enter_context(tc.tile_pool(name="ab", bufs=2))
        tpool = ctx.enter_context(tc.tile_pool(name="tiny", bufs=2))
        pspool = ctx.enter_context(tc.tile_pool(name="ps", bufs=2, space="PSUM"))
        bcpool = ctx.enter_context(tc.tile_pool(name="bc", bufs=2, space="PSUM"))

        # constants
        ones_col = consts.tile([h, 1], F32, tag="ones_col")
        nc.vector.memset(ones_col[:], 1.0)
        ones_row = consts.tile([1, h], F32, tag="ones_row")
        nc.vector.memset(ones_row[:], 1.0)
        eps3 = consts.tile([1, 3], F32, tag="eps3")
        nc.vector.memset(eps3[:, 0:1], EPS)
        nc.vector.memset(eps3[:, 1:2], 0.0)
        nc.vector.memset(eps3[:, 2:3], EPS)
        zero_col = consts.tile([h, 1], F32, tag="zero_col")
        nc.vector.memset(zero_col[:], 0.0)
        # gamma/beta ride the otherwise-idle ACT HWDGE ring so the first
        # z load is the first transfer on the SP ring
        gsb = consts.tile([1, 4], F32, tag="gsb")
        nc.scalar.dma_start(gsb[:], g_ap[:])
        bsb = consts.tile([1, 2], F32, tag="bsb")
        nc.scalar.dma_start(bsb[:], be_ap[:])

        n_dma = 0
        deferred = []  # (dst, tile) store pairs held to the kernel end
        chans = [cc for _ in range(repeat) for cc in range(c_loc)]
        for idx, c in enumerate(chans):
            hb_c = (last_half_b if (last_half_b and idx == len(chans) - 1)
                    else half_b)
            half_b_c, n_half_c, qb_c, lb_c = tiling(hb_c)
            st = stpool.tile([h, 5 * n_half_c], F32, tag="st")
            ps_acc = None
            if acc_reduce:
                ps_acc = pspool.tile([1, 5], F32, tag="psa")
            z_tiles = []
            # ---- pass 1: load + statistics -------------------------------
            for hf in range(n_half_c):
                # held subtiles of the first channel live in rpool until the
                # kernel end (in-place mode: the z tile becomes the o tile)
                zhold = (inplace and reserve and idx == 0
                         and hf >= n_half_c - reserve)
                zt = (rpool if zhold else zpool).tile(
                    [h, half_b_c, w, 2], F32, tag="zr" if zhold else "zt")
                src = z_ap[:, c].transpose([1, 0, 2, 3])[
                    :, hf * half_b_c:(hf + 1) * half_b_c]
                if dma_block:
                    # one transfer per dma_block batch rows: each is a fully
                    # contiguous DRAM run, issued in sequential DRAM order
                    for bb in range(0, half_b_c, dma_block):
                        ls = slice(bb, bb + dma_block)
                        load_cycle[n_dma % len(load_cycle)].dma_start(
                            zt[:, ls], src[:, ls])
                        n_dma += 1
                else:
                    for l in range(ld_split):
                        ls = slice(l * lb_c, (l + 1) * lb_c)
                        load_cycle[n_dma % len(load_cycle)].dma_start(
                            zt[:, ls], src[:, ls])
                        n_dma += 1
                z_tiles.append(zt)
                if probe == "dma":
                    continue
                r_idx, r_hf = remit if remit else (len(chans) - 1,
                                                  n_half_c - 1)
                if ((reserve or defer_prev or defer_prev2)
                        and idx == r_idx and hf == r_hf):
                    # Release the deferred first-channel stores.  Emission
                    # position controls the conservative cross-engine wait
                    # the framework attaches, i.e. when the transfer lands;
                    # tuned so it fills the tail gap after the final load.
                    for ddst, dot in deferred:
                        nc.sync.dma_start(ddst, dot[:])
                        n_dma += 1
                    deferred = []
                z0 = zt[:, :, :, 0]
                z1 = zt[:, :, :, 1]
                o5 = 5 * hf
                ga = gapool.tile([h, half_b_c, w], F32, tag="ga")
                tail = (idx >= len(chans) - split_stats_last
                        and hf == n_half_c - 1)
                if s_dve or (stats_alt and hf % 2 == 1):
                    # S0/S1 on DVE tensor_scalar copy+accum (2x perf mode:
                    # ~0.55 ns/elem vs 0.92 on ACT).  stats_alt: alternate
                    # subtiles so the channel's stats finish ~equally early
                    # on both engines (shortens the stats->apply latency).
                    gd2 = gdpool.tile([h, half_b_c, w], F32, tag="gd")
                    nc.vector.tensor_scalar(
                        out=gd2[:], in0=z0, scalar1=1.0, scalar2=0.0,
                        op0=OP.mult, op1=OP.add,
                        accum_out=st[:, o5 + 0:o5 + 1])
                    nc.vector.tensor_scalar(
                        out=gd2[:], in0=z1, scalar1=1.0, scalar2=0.0,
                        op0=OP.mult, op1=OP.add,
                        accum_out=st[:, o5 + 1:o5 + 2])
                elif tail:
                    # last-loaded subtile: split the 4 accumulations across
                    # ACT and DVE so the exposed tail chain halves.
                    # ts_split uses tensor_scalar (2x DVE perf mode) instead
                    # of scalar_tensor_tensor (1x).
                    gd2 = gdpool.tile([h, half_b_c, w], F32, tag="gd")
                    if ts_split:
                        # two-op form: walrus requires a 2nd op for
                        # TensorScalarPtrReduce (accum) variants
                        nc.vector.tensor_scalar(
                            out=gd2[:], in0=z0, scalar1=1.0, scalar2=0.0,
                            op0=OP.mult, op1=OP.add,
                            accum_out=st[:, o5 + 0:o5 + 1])
                        nc.vector.tensor_scalar(
                            out=gd2[:], in0=z1, scalar1=1.0, scalar2=0.0,
                            op0=OP.mult, op1=OP.add,
                            accum_out=st[:, o5 + 1:o5 + 2])
                    else:
                        nc.vector.scalar_tensor_tensor(
                            out=gd2[:], in0=z0, scalar=1.0, in1=z1,
                            op0=OP.mult, op1=OP.bypass,
                            accum_out=st[:, o5 + 0:o5 + 1])
                        nc.vector.scalar_tensor_tensor(
                            out=gd2[:], in0=z1, scalar=1.0, in1=z0,
                            op0=OP.mult, op1=OP.bypass,
                            accum_out=st[:, o5 + 1:o5 + 2])
                else:
                    nc.scalar.activation(ga[:], z0, AF.Copy,
                                         accum_out=st[:, o5 + 0:o5 + 1])
                    nc.scalar.activation(ga[:], z1, AF.Copy,
                                         accum_out=st[:, o5 + 1:o5 + 2])
                nc.scalar.activation(ga[:], z0, AF.Square,
                                     accum_out=st[:, o5 + 2:o5 + 3])
                nc.scalar.activation(ga[:], z1, AF.Square,
                                     accum_out=st[:, o5 + 4:o5 + 5])
                # Q01 = sum(z0*z1): out = (z0 bypass s) mult z1, accum = sum.
                # (tensor_tensor_reduce crashes this runtime; this is the
                # hardware-verified equivalent.)  q01_pool shifts it to the
                # mostly-idle GPSIMD engine for the channels just before the
                # final one, unloading DVE's tail cascade (the final
                # channel's Q01 stays on DVE: Pool latency would lengthen
                # the exposed tail chain).
                gd = gdpool.tile([h, half_b_c, w], F32, tag="gd")
                q01_eng = (nc.gpsimd if (idx >= len(chans) - 1 - q01_pool
                                         and idx != len(chans) - 1)
                           else nc.vector)
                q01_eng.scalar_tensor_tensor(
                    out=gd[:], in0=z0, scalar=0.0, in1=z1,
                    op0=OP.bypass, op1=OP.mult,
                    accum_out=st[:, o5 + 3:o5 + 4])
                if acc_reduce and probe == "full":
                    # per-subtile partition-reduce, PSUM-accumulated: the
                    # first matmul overlaps the next subtile's load, and the
                    # 2x2 chain loses its S-combine step
                    nc.tensor.matmul(ps_acc[:], lhsT=ones_col[:],
                                     rhs=st[:, o5:o5 + 5],
                                     start=(hf == 0),
                                     stop=(hf == n_half_c - 1))

            if probe in ("dma", "stats"):
                # store straight from the input tiles (timing probe only)
                for hf in range(n_half_c):
                    dst = o_ap[:, c].transpose([1, 0, 2, 3])[
                        :, hf * half_b_c:(hf + 1) * half_b_c]
                    store_cycle[n_dma % len(store_cycle)].dma_start(
                        dst, z_tiles[hf][:])
                    n_dma += 1
                continue

            # ---- partition-reduce the stats with a ones-matmul -----------
            if not acc_reduce:
                ps = pspool.tile([1, 5 * n_half_c], F32, tag="ps")
                nc.tensor.matmul(ps[:], lhsT=ones_col[:], rhs=st[:],
                                 start=True, stop=True)

            # ---- tiny per-channel 2x2 math on partition 0 ----------------
            # T layout (offsets into [1, 64]):
            #  0:5 S | 5:7 mu | 7:10 prods | 13:16 cov | 16 d1 17 d2 18 det
            #  19 s | 20 tr | 21 tr2s | 22 t | 23:26 numer | 26 dsn1 27 dsn2
            #  28 dsn | 29 rdn | 30 f | 31 fn | 32:36 W | 36:40 tmp
            #  40:46 AB = (A00 A01 A10 A11 b0 b1) | 48:52 prod4 | 52:54 ps2
            #  54:64 spare (S-combine accumulators) | 64:64+5n raw psum copy
            T = tpool.tile([1, 112], F32, tag="T")
            v = nc.vector

            def tt(dst, a, bb, op):
                v.tensor_tensor(out=dst, in0=a, in1=bb, op=op)

            # PSUM row -> SBUF (HW allows only one PSUM operand per inst)
            S = T[:, 0:5]
            if acc_reduce:
                # subtile groups were already summed in PSUM by the
                # accumulating matmuls; a single 5-wide copy lands S
                v.tensor_copy(S, ps_acc[:])
            else:
                P = T[:, 64:64 + 5 * n_half_c]
                v.tensor_copy(P, ps[:])

            # S = sum of per-subtile stats groups
            if acc_reduce:
                pass
            elif n_half_c == 1:
                v.tensor_copy(S, P[:, 0:5])
            elif n_half_c == 2:
                tt(S, P[:, 0:5], P[:, 5:10], OP.add)
            else:
                acc = [T[:, 54:59], T[:, 59:64]]
                tt(acc[0], P[:, 0:5], P[:, 5:10], OP.add)
                cur = 0
                for k in range(2, n_half_c):
                    dst = S if k == n_half_c - 1 else acc[1 - cur]
                    tt(dst, acc[cur], P[:, 5 * k:5 * k + 5], OP.add)
                    cur = 1 - cur
            v.tensor_scalar(out=T[:, 5:7], in0=T[:, 0:2], scalar1=inv_n,
                            scalar2=None, op0=OP.mult)
            # prods = (mu0^2, mu0*mu1, mu1^2)
            tt(T[:, 7:9], T[:, 5:7], T[:, 5:6].broadcast_to([1, 2]), OP.mult)
            tt(T[:, 9:10], T[:, 6:7], T[:, 6:7], OP.mult)
            # cov = Q*inv_n - prods + eps*I
            v.scalar_tensor_tensor(out=T[:, 10:13], in0=T[:, 2:5], scalar=inv_n,
                                   in1=T[:, 7:10], op0=OP.mult, op1=OP.subtract)
            tt(T[:, 13:16], T[:, 10:13], eps3[:, 0:3], OP.add)
            # det & s = sqrt(det)
            tt(T[:, 16:17], T[:, 13:14], T[:, 15:16], OP.mult)
            tt(T[:, 17:18], T[:, 14:15], T[:, 14:15], OP.mult)
            tt(T[:, 18:19], T[:, 16:17], T[:, 17:18], OP.subtract)
            nc.scalar.activation(T[:, 19:20], T[:, 18:19], AF.Sqrt)
            # t = sqrt(trace + 2s)
            tt(T[:, 20:21], T[:, 13:14], T[:, 15:16], OP.add)
            v.scalar_tensor_tensor(out=T[:, 21:22], in0=T[:, 19:20], scalar=2.0,
                                   in1=T[:, 20:21], op0=OP.mult, op1=OP.add)
            nc.scalar.activation(T[:, 22:23], T[:, 21:22], AF.Sqrt)
            # numer = (c00+s, c01, c11+s);  W = t/det(numer) * adj(numer)
            tt(T[:, 23:26:2], T[:, 13:16:2], T[:, 19:20].broadcast_to([1, 2]),
               OP.add)
            v.tensor_copy(T[:, 24:25], T[:, 14:15])
            tt(T[:, 26:27], T[:, 23:24], T[:, 25:26], OP.mult)
            tt(T[:, 27:28], T[:, 24:25], T[:, 24:25], OP.mult)
            tt(T[:, 28:29], T[:, 26:27], T[:, 27:28], OP.subtract)
            v.reciprocal(T[:, 29:30], T[:, 28:29])
            tt(T[:, 30:31], T[:, 22:23], T[:, 29:30], OP.mult)
            v.tensor_scalar(out=T[:, 31:32], in0=T[:, 30:31], scalar1=-1.0,
                            scalar2=None, op0=OP.mult)
            # W = (W00, W01, W10, W11) = (f*n2, fn*n1, fn*n1, f*n0)
            tt(T[:, 32:33], T[:, 25:26], T[:, 30:31], OP.mult)
            tt(T[:, 33:34], T[:, 24:25], T[:, 31:32], OP.mult)
            v.tensor_copy(T[:, 34:35], T[:, 33:34])
            tt(T[:, 35:36], T[:, 23:24], T[:, 30:31], OP.mult)
            # A = gamma @ W  (row i = g_i0 * Wtop + g_i1 * Wbot)
            v.tensor_scalar(out=T[:, 36:38], in0=T[:, 32:34],
                            scalar1=gsb[:, 0:1], scalar2=None, op0=OP.mult)
            v.scalar_tensor_tensor(out=T[:, 40:42], in0=T[:, 34:36],
                                   scalar=gsb[:, 1:2], in1=T[:, 36:38],
                                   op0=OP.mult, op1=OP.add)
            v.tensor_scalar(out=T[:, 38:40], in0=T[:, 32:34],
                            scalar1=gsb[:, 2:3], scalar2=None, op0=OP.mult)
            v.scalar_tensor_tensor(out=T[:, 42:44], in0=T[:, 34:36],
                                   scalar=gsb[:, 3:4], in1=T[:, 38:40],
                                   op0=OP.mult, op1=OP.add)
            # b = beta - A @ mu
            tt(T[:, 48:50], T[:, 40:42], T[:, 5:7], OP.mult)
            tt(T[:, 50:52], T[:, 42:44], T[:, 5:7], OP.mult)
            tt(T[:, 52:54], T[:, 48:52:2], T[:, 49:52:2], OP.add)
            tt(T[:, 44:46], bsb[:, 0:2], T[:, 52:54], OP.subtract)

            # ---- broadcast (A, b) to all partitions via K=1 matmul -------
            bc = bcpool.tile([h, 6], F32, tag="bc")
            nc.tensor.matmul(bc[:], lhsT=ones_row[:], rhs=T[:, 40:46],
                             start=True, stop=True)
            if ab_psum:
                # apply ops read the per-partition (A, b) scalars straight
                # from the PSUM broadcast (their only PSUM operand), saving
                # the SBUF copy on the chain critical path
                ab = bc
            else:
                ab = abpool.tile([h, 6], F32, tag="ab")
                v.tensor_copy(ab[:], bc[:])

            # ---- pass 2: apply o = A z + b, store ------------------------
            osplit_c = (osplit_last
                        if (osplit_last and idx >= len(chans) - osl_chans)
                        else osplit)
            qb_c = half_b_c // osplit_c
            for hf in range(n_half_c):
                zt = z_tiles[hf]
                for q in range(osplit_c):
                    bs = slice(q * qb_c, (q + 1) * qb_c)
                    z0 = zt[:, bs, :, 0]
                    z1 = zt[:, bs, :, 1]
                    u0 = upool.tile([h, qb_c, w], F32, tag="u")
                    u1 = upool.tile([h, qb_c, w], F32, tag="u")
                    o1p = (not inplace
                           and (idx >= len(chans) - 1 - o1_pool
                                and idx != len(chans) - 1
                                if o1_pool_skipf
                                else idx >= len(chans) - o1_pool))
                    if fuse_g0 and hf == 0 and q == 0:
                        # first group of an o1_pool channel: the Pool o1
                        # detour adds ~0.6us of latency the first store
                        # cannot hide, so keep it fused on DVE
                        o1p = False

                    # u0 = A01*z1 + b0 (ACT; DVE for the tail channels so
                    # the kernel end is not gated on the busiest engine)
                    def emit_u0():
                        if (idx >= len(chans) - dve_u0_last
                                or (dve_u0_o1p and o1p)
                                or (du0_g0 and idx == len(chans) - 1
                                    and hf == 0 and q == 0)):
                            nc.vector.tensor_scalar(
                                out=u0[:], in0=z1, scalar1=ab[:, 1:2],
                                scalar2=ab[:, 4:5], op0=OP.mult, op1=OP.add)
                        else:
                            nc.scalar.activation(u0[:], z1, AF.Identity,
                                                 bias=ab[:, 4:5],
                                                 scale=ab[:, 1:2])

                    def emit_u1():
                        if u1_act or idx >= len(chans) - act_u1_last:
                            # DVE is the apply-chain bottleneck (u1 + 2
                            # fuses); produce u1 on ACT instead
                            nc.scalar.activation(u1[:], z1, AF.Identity,
                                                 bias=ab[:, 5:6],
                                                 scale=ab[:, 3:4])
                        else:
                            nc.vector.tensor_scalar(
                                out=u1[:], in0=z1, scalar1=ab[:, 3:4],
                                scalar2=ab[:, 5:6], op0=OP.mult, op1=OP.add)

                    emit_u0()
                    emit_u1()
                    if inplace:
                        # Apply writes back into the z tile: z1 is dead once
                        # u0/u1 are computed, so o1 overwrites it; z0 is dead
                        # once o1 has read it, so o0 overwrites z0 last.
                        hold = (reserve and idx == 0
                                and hf >= n_half_c - reserve)
                        ot = zt[:, bs]
                        nc.vector.scalar_tensor_tensor(
                            out=zt[:, bs, :, 1], in0=z0, scalar=ab[:, 2:3],
                            in1=u1[:], op0=OP.mult, op1=OP.add)
                        nc.vector.scalar_tensor_tensor(
                            out=zt[:, bs, :, 0], in0=z0, scalar=ab[:, 0:1],
                            in1=u0[:], op0=OP.mult, op1=OP.add)
                    else:
                        g_idx = hf * osplit + q
                        hold = (reserve and idx == 0
                                and g_idx >= n_half_c * osplit - reserve)
                        ot = (rpool if hold else opool).tile(
                            [h, qb_c, w, 2], F32, tag="rt" if hold else "ot")
                        if o1p:
                            # o1 = (A10*z0) + u1 with the final add on the
                            # mostly-idle GPSIMD engine; same expression tree
                            # as the fused stt so numerics are unchanged.
                            # v1_act computes A10*z0 on ACT (Identity with a
                            # zero bias) instead of a DVE tensor_scalar.
                            v1 = upool.tile([h, qb_c, w], F32, tag="u")
                            if v1_act:
                                nc.scalar.activation(
                                    v1[:], z0, AF.Identity,
                                    bias=zero_col[:, 0:1],
                                    scale=ab[:, 2:3])
                            else:
                                nc.vector.tensor_scalar(
                                    out=v1[:], in0=z0, scalar1=ab[:, 2:3],
                                    scalar2=None, op0=OP.mult)
                            nc.gpsimd.tensor_tensor(
                                out=ot[:, :, :, 1], in0=v1[:], in1=u1[:],
                                op=OP.add)
                        else:
                            nc.vector.scalar_tensor_tensor(
                                out=ot[:, :, :, 1], in0=z0, scalar=ab[:, 2:3],
                                in1=u1[:], op0=OP.mult, op1=OP.add)
                        nc.vector.scalar_tensor_tensor(
                            out=ot[:, :, :, 0], in0=z0, scalar=ab[:, 0:1],
                            in1=u0[:], op0=OP.mult, op1=OP.add)
                    dst = o_ap[:, c].transpose([1, 0, 2, 3])[
                        :, hf * half_b_c + q * qb_c:hf * half_b_c + (q + 1) * qb_c]
                    ssrc = ot if inplace else ot[:]
                    if not hold and defer_prev and idx == len(chans) - 2:
                        # Defer the second-to-last channel's trailing stores:
                        # emitted after the final channel's loads so their
                        # SemWaits cannot block the last load decodes on the
                        # SP sequencer, yet their (ready) data fills the DMA
                        # pipe the moment the final load transfer ends.
                        g_idx = hf * osplit_c + q
                        hold = g_idx >= n_half_c * osplit_c - defer_prev
                    if not hold and defer_prev2 and idx == len(chans) - 3:
                        # same for the third-to-last channel's trailing
                        # stores: their SWDGE desc-gen latency (~2.1us) was
                        # exposing a mid-tail bubble; the sync-ring deferred
                        # path re-queues them ~0.8us earlier
                        g_idx = hf * osplit_c + q
                        hold = g_idx >= n_half_c * osplit_c - defer_prev2
                    if hold:
                        assert not inplace, "reserve requires tile-held o"
                        deferred.append((dst, ot))
                        continue
                    if dma_block:
                        for bb in range(0, qb_c, dma_block):
                            os_ = slice(bb, bb + dma_block)
                            store_cycle[n_dma % len(store_cycle)].dma_start(
                                dst[:, os_], ot[:, os_])
                            n_dma += 1
                    else:
                        # last channel: loads are done, the SP HWDGE ring is
                        # free and issues ~2us faster than SWDGE descriptors
                        if idx >= len(chans) - tail_sync_stores:
                            nc.sync.dma_start(dst, ssrc)
                        else:
                            store_cycle[n_dma % len(store_cycle)].dma_start(
                                dst, ssrc)
                        n_dma += 1

    nc.compile()
    return nc


_PROGRAM_CACHE = {}


def _get_program(key):
    if key not in _PROGRAM_CACHE:
        _PROGRAM_CACHE[key] = build_program(*key[:4], **dict(key[4]))
    return _PROGRAM_CACHE[key]


def kernel(z, gamma, beta):
    z = np.asarray(z, dtype=np.float32)
    gamma = np.asarray(gamma, dtype=np.float32)
    beta = np.asarray(beta, dtype=np.float32)
    assert z.shape == (B, C, H, W, 2), z.shape

    from concourse.bass_utils import run_bass_kernel_spmd

    nc = _get_program((B, C_LOC, H, W, tuple(sorted(CFG.items()))))

    g4 = np.ascontiguousarray(gamma.reshape(1, 4))
    b2 = np.ascontiguousarray(beta.reshape(1, 2))
    in_maps = []
    for k in range(N_CORES):
        shard = np.ascontiguousarray(z[:, k * C_LOC:(k + 1) * C_LOC])
        in_maps.append({"z": shard, "gamma": g4, "beta": b2})

    res = run_bass_kernel_spmd(nc, in_maps, list(range(N_CORES)))
    out = np.concatenate([res.results[k]["out"] for k in range(N_CORES)], axis=1)
    return out

